# revision 1
# baseline (speedup 1.0000x reference)
"""Distributed multi-head attention kernel for one TRN2 chip (8 NeuronCores).

Problem: B=2, S=2048, D=1024, H=16 heads (head_dim 64), torch-style
Linear QKV projections + softmax attention + out projection.

Sharding: tensor-parallel over heads, 2 heads per core (all 8 cores see the
full batch).  Each core:
  1. computes qT/kT/vT = (x @ W.T + b).T for its 2 heads (E=128 local dims),
  2. runs softmax attention for its (2 heads x 2 batches) fully locally
     (scores computed transposed [k, q] so softmax-sum comes free via an
     appended ones-column in V during the PV matmul),
  3. AllToAll redistributes attention outputs so core j owns query rows
     [j*512:(j+1)*512] of the flattened [B*S, D] activations with all 16
     heads present,
  4. local out-projection (x @ Wo.T + bo) for its 512 rows.
Host reassembles the 8 row-blocks into the [B, S, D] output.

Scheduling: the kernel is paced by TensorEngine cycles under the HAM
power-duty governor, so the schedule minimizes structural serialization:
unit 0's scores start after only k0+q0+q1 land (remaining batch-0
projection DMA triggers/computes interleave between its score c-tiles),
and batch-1 projections and batch-0's out-projection spread evenly
across attention units as fillers.

Compute dtype: bfloat16 on the matmuls (fp32 accumulation in PSUM), exp in
fp32->bf16 on the scalar engine, normalization in fp32.
"""

import numpy as np

B = 2
S = 2048
D = 1024
H = 16
DH = 64
N_CORES = 8
HPC = H // N_CORES  # heads per core = 2
E = HPC * DH  # local head dims = 128
ROWS = B * S // N_CORES  # output rows per core = 512
NT = B * S  # total tokens = 4096
DCH = D // 128  # d-model chunks of 128 = 8
SCALE = 1.0 / np.sqrt(DH)

_CACHE = {}


def _bf16(x):
    import ml_dtypes

    return np.ascontiguousarray(x).astype(ml_dtypes.bfloat16)


def _build():
    """Build + compile the SPMD Bass graph (identical on all 8 cores)."""
    from concourse import bacc, tile, mybir

    bf16 = mybir.dt.bfloat16
    f32 = mybir.dt.float32
    AF = mybir.ActivationFunctionType

    nc = bacc.Bacc("TRN2", target_bir_lowering=False, debug=False, num_devices=N_CORES)

    # ---- I/O -----------------------------------------------------------
    # activations, pre-transposed AND pre-tiled on host:
    # [NT//512, 128, DCH, 512]: element (t, p, d, c) = x[t*512 + c, d*128 + p]
    xq = nc.dram_tensor("xq", [NT // 512, 128, DCH, 512], bf16, kind="ExternalInput")
    xk = nc.dram_tensor("xk", [NT // 512, 128, DCH, 512], bf16, kind="ExternalInput")
    xv = nc.dram_tensor("xv", [NT // 512, 128, DCH, 512], bf16, kind="ExternalInput")
    # weights, pre-transposed/sliced on host: [128, DCH, E]:
    # (p, d, e) = W[head_slice][e_global, d*128+p] (scale folded into wq)
    wq = nc.dram_tensor("wq", [128, DCH, E], bf16, kind="ExternalInput")
    wk = nc.dram_tensor("wk", [128, DCH, E], bf16, kind="ExternalInput")
    wv = nc.dram_tensor("wv", [128, DCH, E], bf16, kind="ExternalInput")
    # full Wo.T: (p, d, e) = Wo[e, d*128+p]
    wo = nc.dram_tensor("wo", [128, DCH, D], bf16, kind="ExternalInput")
    # biases: per-partition columns (scale folded into bq)
    bq = nc.dram_tensor("bq", [128, 1], f32, kind="ExternalInput")
    bk = nc.dram_tensor("bk", [128, 1], f32, kind="ExternalInput")
    bv = nc.dram_tensor("bv", [128, 1], f32, kind="ExternalInput")
    # bo replicated across partitions
    bo = nc.dram_tensor("bo", [128, D], f32, kind="ExternalInput")
    out = nc.dram_tensor("out", [ROWS, D], f32, kind="ExternalOutput")

    SKT = S // 128  # 16 k-tiles per batch

    with tile.TileContext(nc) as tc:
        with (
            tc.tile_pool(name="dram", bufs=1, space="DRAM") as dram,
            tc.tile_pool(name="wpool", bufs=1) as wpool,
            tc.tile_pool(name="xs", bufs=6) as xs_pool,
            tc.tile_pool(name="qk", bufs=1) as qk_pool,
            tc.tile_pool(name="vpool", bufs=1) as v_pool,
            # PSUM budget (8 banks): scores/proj/outproj share 2x[128,1024]
            # slots = 4 banks, double-buffered PV accumulators = 4 banks
            tc.tile_pool(name="ps", bufs=2, space="PSUM") as ps_pool,
            tc.tile_pool(name="pso", bufs=2, space="PSUM") as pso_pool,
            tc.tile_pool(name="ex", bufs=34) as e_pool,
            tc.tile_pool(name="norm", bufs=2) as n_pool,
            tc.tile_pool(name="ao", bufs=1) as ao_pool,
            tc.tile_pool(name="outp", bufs=2) as out_pool,
        ):
            a2a_in = [dram.tile([N_CORES, E, ROWS // 2], bf16, name=f"a2ai{b}")
                      for b in range(B)]
            a2a_out = [dram.tile([N_CORES, E, ROWS // 2], bf16, name=f"a2ao{b}")
                       for b in range(B)]
            warm_in = dram.tile([N_CORES, 128], bf16, name="warm_in")
            warm_out = dram.tile([N_CORES, 128], bf16, name="warm_out")

            # ---- load weights / biases --------------------------------
            wq_sb = wpool.tile([128, DCH, E], bf16, tag="wq")
            wk_sb = wpool.tile([128, DCH, E], bf16, tag="wk")
            wv_sb = wpool.tile([128, DCH, E], bf16, tag="wv")
            wo_sb = wpool.tile([128, DCH, D], bf16, tag="wo")
            bq_sb = wpool.tile([128, 1], f32, tag="bq")
            bk_sb = wpool.tile([128, 1], f32, tag="bk")
            bv_sb = wpool.tile([128, 1], f32, tag="bv")
            bo_sb = wpool.tile([128, D], f32, tag="bo")
            ident = wpool.tile([128, 128], bf16, tag="ident")
            import ml_dtypes

            ident_dram = nc.inline_tensor(
                np.eye(128, dtype=ml_dtypes.bfloat16), name="ident_c"
            )
            # only ident/wq precede the head-critical x triggers; the
            # rest of the weights are emitted after them (see main flow)
            # so k0/q0/q1 transfers start as early as possible
            nc.sync.dma_start(ident[:], ident_dram[:])
            nc.scalar.dma_start(wq_sb[:], wq[:])

            # ---- persistent activation tiles --------------------------
            qT = [qk_pool.tile([128, S], bf16, tag=f"qT{b}", name=f"qT{b}")
                  for b in range(B)]
            kT = [qk_pool.tile([128, S], bf16, tag=f"kT{b}", name=f"kT{b}")
                  for b in range(B)]
            vT = [qk_pool.tile([128, S], bf16, tag=f"vT{b}", name=f"vT{b}")
                  for b in range(B)]
            # v natural, augmented with ones col: [s-part, kt, h, 65]
            v_sb = [v_pool.tile([128, SKT, HPC, DH + 1], bf16, tag=f"v{b}",
                                name=f"v{b}") for b in range(B)]

            # PE warmup: a short burst of dummy transposes while weights
            # stream in, so the clock ramps before the first projections.
            wps = ps_pool.tile([128, 512], bf16, tag="ps", name="warm")
            for _ in range(12):
                nc.tensor.transpose(wps[:, 0:128], ident[:], ident[:])

            class ProjTile:
                """One 512-column projection tile with separately emittable
                DMA-trigger and compute (8 matmuls + bias) parts, so head
                tiles' triggers never queue behind compute-dependent
                instructions."""

                def __init__(self, xdram, w_sb, bias_sb, out_tile, tg, st, q):
                    self.xdram, self.w_sb, self.bias_sb = xdram, w_sb, bias_sb
                    self.out_tile, self.tg, self.st, self.q = out_tile, tg, st, q
                    self.xt = None

                def dma(self):
                    self.xt = xs_pool.tile([128, DCH, 512], bf16, tag="xt")
                    self.q.dma_start(self.xt[:], self.xdram[self.tg])

                def compute(self):
                    ps = ps_pool.tile([128, 512], f32, tag="ps", name="ps_proj")
                    for d in range(DCH):
                        nc.tensor.matmul(
                            ps[:], self.w_sb[:, d, :], self.xt[:, d, :],
                            start=(d == 0), stop=(d == DCH - 1),
                        )
                    nc.vector.tensor_scalar_add(
                        self.out_tile[:, self.st * 512:(self.st + 1) * 512],
                        ps[:], self.bias_sb[:],
                    )

                def both(self):
                    self.dma()
                    self.compute()

            def proj_pairs(b, queues):
                # queues: either a short list cycled round-robin, or an
                # explicit 12-entry list indexed [q0..q3, k0..k3, v0..v3]
                th = []
                n = 0
                for xdram, w_sb, bias_sb, out_t in (
                    (xq, wq_sb, bq_sb, qT[b]),
                    (xk, wk_sb, bk_sb, kT[b]),
                    (xv, wv_sb, bv_sb, vT[b]),
                ):
                    for st in range(4):
                        th.append(ProjTile(xdram, w_sb, bias_sb, out_t,
                                           b * 4 + st, st, queues[n % len(queues)]))
                        n += 1
                return th

            def v_finish(b, c0=0, c1=SKT):
                # natural-layout V (with ones column) via PE transposes
                if c0 == 0:
                    nc.vector.memset(v_sb[b][:, :, :, DH:DH + 1], 1.0)
                for c in range(c0, c1):
                    pst = ps_pool.tile([128, 512], bf16, tag="ps", name="pst")
                    nc.tensor.transpose(
                        pst[:, 0:128], vT[b][:, c * 128:(c + 1) * 128], ident[:]
                    )
                    nc.vector.tensor_copy(
                        v_sb[b][:, c, :, 0:DH],
                        pst[:, 0:128].rearrange("p (h d) -> p h d", h=HPC),
                    )

            # ---- attention (head-sequential, fillers interleaved) -----
            exs_store = {}
            aoT = [ao_pool.tile([64, S], bf16, tag=f"aoT{h}",
                                name=f"aoT{h}") for h in range(HPC)]

            def phase_a(b, qh, h, co_emits=None):
                q0 = qh * 1024
                p0 = h * 64
                exs = []
                for c in range(SKT):
                    if co_emits and c in co_emits:
                        for th in co_emits[c]:
                            th()
                    pss = ps_pool.tile([128, 1024], f32, tag="ps",
                                       name="pss")
                    for half in range(2):
                        nc.tensor.matmul(
                            pss[:, half * 512:(half + 1) * 512],
                            kT[b][p0:p0 + 64, c * 128:(c + 1) * 128],
                            qT[b][p0:p0 + 64,
                                  q0 + half * 512:q0 + half * 512 + 512],
                            start=True, stop=True,
                            tile_position=(p0, 0),
                        )
                    ex = e_pool.tile([128, 1024], bf16, tag="ex",
                                     name=f"ex{c}")
                    nc.scalar.activation(ex[:], pss[:], AF.Exp)
                    exs.append(ex)
                exs_store[(b, qh, h)] = exs

            def phase_b(b, qh, h, filler):
                q0 = qh * 1024
                exs = exs_store.pop((b, qh, h))
                pso = pso_pool.tile([65, 1024], f32, tag="pso",
                                    name=f"pso{b}_{qh}_{h}")
                for c in range(SKT):
                    for sub in range(2):
                        nc.tensor.matmul(
                            pso[:, sub * 512:(sub + 1) * 512],
                            v_sb[b][:, c, h, :],
                            exs[c][:, sub * 512:(sub + 1) * 512],
                            start=(c == 0), stop=(c == SKT - 1),
                        )
                # PE-only filler work rides the exp shadow
                for th in filler:
                    th()
                # normalize.  Mid-stream units use the DVE reciprocal
                # (~7.9us, but off the critical path; the ACT engine is
                # busy pacing each unit with 16 exps, and custom-DVE
                # approx ops lack their uop table under this runtime).
                # The LAST unit's normalize is on the tail critical path
                # and ACT is idle by then, so compute 1/x there as
                # exp(-ln(x)) on ACT (Exp and Ln share a table set).
                rc = n_pool.tile([1, 1024], f32, tag="rc", bufs=1)
                if (b, qh, h) == (B - 1, 1, HPC - 1):
                    lg = n_pool.tile([1, 1024], f32, tag="lg", bufs=1)
                    nc.scalar.activation(lg[:], pso[64:65, :], AF.Ln)
                    nc.scalar.activation(rc[:], lg[:], AF.Exp, scale=-1.0)
                else:
                    nc.vector.reciprocal(rc[:], pso[64:65, :])
                bc = n_pool.tile([64, 1024], f32, tag="bc")
                nc.gpsimd.partition_broadcast(bc[:], rc[:])
                nc.vector.tensor_mul(
                    aoT[h][:, q0:q0 + 1024], pso[0:64, :], bc[:]
                )
                # ship this (q-half, head) slice immediately: q-half qh
                # covers shards 4qh..4qh+3 (q rows j*256..)
                nc.scalar.dma_start(
                    a2a_in[b][qh * 4:(qh + 1) * 4,
                              h * 64:(h + 1) * 64, :]
                    .transpose([1, 0, 2]),
                    aoT[h][:, q0:q0 + 1024]
                    .rearrange("p (j c) -> p j c", j=4),
                )

            def attention_all(units, fillers):
                # one software-pipelined stream across BOTH batches: phase A
                # of unit n+1 before phase B of unit n, including across the
                # batch boundary (kills the ~24us ACT hole there). batch-0's
                # AllToAll fires right after its last unit ships.
                # phase_a(units[0]) must have been emitted by the caller.
                for i in range(len(units)):
                    if i + 1 < len(units):
                        phase_a(*units[i + 1])
                    phase_b(*units[i], fillers[i])
                    if i == 3:
                        a2a(0)

            def a2a(b):
                nc.gpsimd.collective_compute(
                    "AllToAll",
                    mybir.AluOpType.bypass,
                    replica_groups=[list(range(N_CORES))],
                    ins=[a2a_in[b][:].opt()],
                    outs=[a2a_out[b][:].opt()],
                )

            # ---- out projection (per batch half: 256 rows) -----------
            def outproj_group(b, ao_d, st, half):
                e0 = half * 512
                ps = ps_pool.tile([128, 512], f32, tag="ps", name="ps_out")
                for d in range(DCH):
                    nc.tensor.matmul(
                        ps[:],
                        ao_d[d][:, st * 128:(st + 1) * 128],
                        wo_sb[:, d, e0:e0 + 512],
                        start=(d == 0), stop=(d == DCH - 1),
                    )
                ot = out_pool.tile([128, 512], f32, tag="ot")
                nc.vector.tensor_add(ot[:], ps[:], bo_sb[:, e0:e0 + 512])
                r0 = b * 256 + st * 128
                nc.sync.dma_start(out[r0:r0 + 128, e0:e0 + 512], ot[:])

            def outproj_thunks(b):
                # per-peer-chunk tiles: each 64KB load is an independent
                # dependency, so the first matmuls start as soon as the
                # first chunk lands instead of after the full 0.5MB
                ao_d = [ao_pool.tile([128, ROWS // 2], bf16, tag=f"ao_d{d}",
                                     name=f"ao{b}_d{d}") for d in range(DCH)]
                for d in range(DCH):
                    nc.sync.dma_start(ao_d[d][:], a2a_out[b][d])
                return [
                    lambda st=st, half=half: outproj_group(b, ao_d, st, half)
                    for st in range(2) for half in range(2)
                ]

            # ---- main flow -------------------------------------------
            # Minimal head: trigger k0/q0/q1 DMAs (3MB on three queues),
            # compute those three tiles, then start phase_a(unit 0) with
            # the remaining batch-0 DMA triggers and computes interleaved
            # between its score c-tiles (scores consume one k-tile per 4
            # c-tiles, so k1/k2/k3 computes land just ahead of need).
            units = [(b, qh, h) for b in range(B) for qh in range(2)
                     for h in range(HPC)]
            # explicit head queues (only sync/scalar/gpsimd can trigger
            # DMAs); the warmup collective is emitted AFTER phase_a(u0)
            # below so gpsimd is free at t=0 and the three head-critical
            # tiles k0/q0/q1 stream on three queues in parallel
            p0 = proj_pairs(0, [
                nc.scalar, nc.gpsimd, nc.sync, nc.scalar,   # q0..q3
                nc.sync, nc.gpsimd, nc.scalar, nc.sync,     # k0..k3
                nc.gpsimd, nc.scalar, nc.sync, nc.gpsimd,   # v0..v3
            ])
            for i in (4, 0, 1):   # k0 (sync), q0 (scalar), q1 (gpsimd)
                p0[i].dma()
            # remaining weights ride behind the head-critical transfers
            nc.sync.dma_start(wk_sb[:], wk[:])
            nc.scalar.dma_start(wv_sb[:], wv[:])
            nc.scalar.dma_start(bq_sb[:], bq[:])
            nc.scalar.dma_start(bk_sb[:], bk[:])
            nc.scalar.dma_start(bv_sb[:], bv[:])
            for i in (4, 0, 1):
                p0[i].compute()
            phase_a(*units[0], co_emits={
                0: [p0[5].dma],                      # k1 (gpsimd)
                1: [p0[6].dma],                      # k2 (scalar)
                2: [p0[2].dma],                      # q2 (sync)
                3: [p0[5].compute],                  # k1 (needed c>=4)
                4: [p0[7].dma],                      # k3 (sync)
                5: [p0[3].dma],                      # q3 (scalar)
                7: [p0[6].compute],                  # k2 (needed c>=8)
                8: [p0[8].dma],                      # v0 (gpsimd)
                9: [p0[9].dma],                      # v1 (scalar)
                11: [p0[7].compute],                 # k3 (needed c>=12)
                12: [p0[10].dma],                    # v2 (sync)
                13: [p0[11].dma],                    # v3 (gpsimd)
            })
            # collective warmup: a tiny AllToAll absorbs the first-call
            # ncfw setup cost (~40us) well before a2a(0) needs steady
            # state; emitted here (not at t=0) so its ~10us of gpsimd
            # queue issue time doesn't block the head DMA triggers
            nc.gpsimd.collective_compute(
                "AllToAll",
                mybir.AluOpType.bypass,
                replica_groups=[list(range(N_CORES))],
                ins=[warm_in[:].opt()],
                outs=[warm_out[:].opt()],
            )
            # remaining batch-0 computes ride the unit-0 exp shadow after
            # all 16 score c-tiles (keeping unit 0's PE stream lean)
            for i in (2, 3, 8, 9, 10, 11):
                p0[i].compute()
            v_finish(0)
            # out-projection weights are needed only much later; loading
            # them after the critical projection emission keeps the 2.5MB
            # off the HBM-critical start window
            nc.scalar.dma_start(wo_sb[:], wo[:])
            nc.scalar.dma_start(bo_sb[:], bo[:])
            # batch-1 projections ride inside batch-0 attention units,
            # spread evenly so no single unit's PE+DMA load spikes (power
            # spikes trigger 50%-duty HAM throttle windows). Deadlines:
            # k0-3+q0,q1 by f1 (phase_a(u4) emitted at i=3), v+v_finish
            # by f3 (phase_b(u4)), q2,q3 by f4 (phase_a(u6) at i=5).
            p1 = proj_pairs(1, [nc.gpsimd, nc.sync])
            op0_holder = []

            fillers = [
                [p1[4].both, p1[5].both, p1[0].both],         # k0, k1, q0
                [p1[6].both, p1[7].both, p1[1].both],         # k2, k3, q1
                [p1[8].both, p1[9].both,
                 lambda: v_finish(1, 0, 8)],                  # v0, v1
                [p1[10].both, p1[11].both,
                 lambda: v_finish(1, 8, 16)],                 # v2, v3
                [p1[2].both, p1[3].both],                     # q2, q3
                # issue the outproj(0) DMA loads only (no PE work) after
                # every batch-1 x-tile DMA is on the sync queue, so the
                # a2a_out wait can't block them
                [lambda: op0_holder.extend(outproj_thunks(0))],
                # outproj(0) groups ride the last two units' PE slack
                [lambda: op0_holder[0](), lambda: op0_holder[1]()],
                [lambda: op0_holder[2](), lambda: op0_holder[3]()],
            ]
            attention_all(units, fillers)
            a2a(1)
            for th in outproj_thunks(1):
                th()

    nc.compile()
    return nc


def _prep_inputs(query, key, value, Wq, bq, Wk, bk, Wv, bv, Wo, bo):
    """Host-side sharding/layout. Returns list of 8 per-core input dicts."""
    x_flat = {}
    for name, x in (("xq", query), ("xk", key), ("xv", value)):
        # [B,S,D] -> [NT, D] -> T [D, NT] -> [NT//512, 128, DCH, 512]
        xt = x.reshape(NT, D).T.reshape(DCH, 128, NT // 512, 512)
        x_flat[name] = _bf16(xt.transpose(2, 1, 0, 3))

    wo_l = _bf16(Wo.T.reshape(DCH, 128, D).transpose(1, 0, 2))
    bo_l = np.ascontiguousarray(
        np.broadcast_to(bo.astype(np.float32), (128, D))
    )

    in_maps = []
    for i in range(N_CORES):
        r0 = i * E  # global head-dim slice for this core
        m = dict(x_flat)
        m["wq"] = _bf16(
            (Wq[r0:r0 + E, :] * SCALE).T.reshape(DCH, 128, E).transpose(1, 0, 2)
        )
        m["wk"] = _bf16(Wk[r0:r0 + E, :].T.reshape(DCH, 128, E).transpose(1, 0, 2))
        m["wv"] = _bf16(Wv[r0:r0 + E, :].T.reshape(DCH, 128, E).transpose(1, 0, 2))
        m["wo"] = wo_l
        m["bq"] = np.ascontiguousarray(
            (bq[r0:r0 + E] * SCALE).astype(np.float32).reshape(128, 1)
        )
        m["bk"] = np.ascontiguousarray(bk[r0:r0 + E].astype(np.float32).reshape(128, 1))
        m["bv"] = np.ascontiguousarray(bv[r0:r0 + E].astype(np.float32).reshape(128, 1))
        m["bo"] = bo_l
        in_maps.append(m)
    return in_maps


def _get_nc():
    if "nc" not in _CACHE:
        _CACHE["nc"] = _build()
    return _CACHE["nc"]


def kernel(query, key, value, Wq, bq, Wk, bk, Wv, bv, Wo, bo, _trace=False):
    from concourse import bass_utils

    query = np.asarray(query, np.float32)
    key = np.asarray(key, np.float32)
    value = np.asarray(value, np.float32)
    nc = _get_nc()
    in_maps = _prep_inputs(
        query, key, value,
        np.asarray(Wq, np.float32), np.asarray(bq, np.float32),
        np.asarray(Wk, np.float32), np.asarray(bk, np.float32),
        np.asarray(Wv, np.float32), np.asarray(bv, np.float32),
        np.asarray(Wo, np.float32), np.asarray(bo, np.float32),
    )
    res = bass_utils.run_bass_kernel_spmd(
        nc, in_maps, core_ids=list(range(N_CORES)), trace=_trace
    )
    outf = np.empty((B, S, D), np.float32)
    half = ROWS // 2
    for i in range(N_CORES):
        o = np.asarray(res.results[i]["out"]).astype(np.float32)
        for b in range(B):
            outf[b, i * half:(i + 1) * half] = o[b * half:(b + 1) * half]
    result = outf
    if _trace:
        _CACHE["last_results"] = res
    return result

